# revision 26
# baseline (speedup 1.0000x reference)
"""Cross-attention kernel for Trainium2, 8 NeuronCores, data-parallel over batch.

Reference math per batch b:
    q_proj = q[b] @ Wq;  k_proj = y[b] @ Wk;  v_proj = k_proj @ Wv
    out = softmax(q_proj @ k_proj.T / 32) @ v_proj

Host-side restructure (kills the k-projection entirely, -14% device FLOPs):
    Wqk = Wq @ Wk.T   ->  scores = (q @ Wqk) @ y.T / 32
    Wkv = Wk @ Wv     ->  v_proj = y @ Wkv

Device per core (one batch per core, feature-major "T" layouts, no
on-device transposes):
    g   = q @ (16*Wqk)            fp16 matmul          [d', n] in PSUM (=16g)
    g8  = fp8e4(psum)             ScalarE quantize     (full d)
    v   = y @ Wkv                 fp16 matmul          [m, f] fp16
    S   = y8.T @ g8               fp8 DoubleRow, full 1024-d contraction
    eT  = exp(S / 8192)           ScalarE              [m, n] fp16
    out = (eT.T @ v) / denom      fp16

fp16 matmul runs at the same PE rate as bf16 but carries 10 mantissa
bits, so every non-fp8 stage contributes ~8x less rounding noise than
the bf16 version.  That freed error budget lets the whole 1024-deep
scores contraction run fp8e4 DoubleRow (2x throughput): rel err 1.90e-2
vs the 2e-2 gate, verified bit-close (4e-5) against a host emulator of
the exact quantization chain.  y is quantized on host (scaled by 16 to
dodge the e4m3 subnormal range; the scale folds into exp's 1/8192).

The softmax denominator is computed off the tensor engine: VectorE
sums eT over the 16 key-chunks with an fp16 add chain (during
otherwise-idle windows), leaving one 1-column fp16 matmul per
128-query subblock for the cross-partition sum (the old ones-matmul
version burned ~8us of PE time on 256 interleaved 1-col matmuls, and
an f32 matmul here downclocks the whole PE by 20%).

Phase order per core: g0 S0 g1 S1 V out0 g2 S2 out1 g3 S3 out2 out3.
Deferring the v-projection until after S1 keeps the tensor engine off
the startup DMA critical path (the input stream is HBM-BW-bound for the
first ~40us).  All q-blocks are preloaded (qt bufs=4) and every input
is shipped in exact SBUF layout, striped across the sync+gpsimd DMA
queues in consumption order (the scalar HW queue measured ~4x slower
and hurt the aggregate ramp).  Warmup matmuls ramp the PE clock during
the first DMA.  Output is fp16 (upcast on host); out-subblocks scale fb0 on
ScalarE / fb1 on VectorE so the epilogue overlaps the next group's
matmuls, and the final subblock splits its last scale across both
engines to shorten the end-of-kernel drain chain.
"""

import numpy as np
import ml_dtypes
from contextlib import ExitStack

import concourse.bass as bass
import concourse.tile as tile
from concourse import bacc, mybir
from concourse.bass_utils import run_bass_kernel_spmd

P = 128
F32 = mybir.dt.float32
F16 = mybir.dt.float16
FP8 = mybir.dt.float8e4
E4NP = ml_dtypes.float8_e4m3
F16NP = np.float16

# Problem shapes (hardcoded per contract)
B = 8
NQ = 2048
NK = 2048
D = 1024   # in_q_dim == in_dim == hid_q == out_dim
F = 1024

YSCALE = 16.0   # host folds into y8;   |16*y|  < ~90  (e4m3 max 240)
GSCALE = 16.0   # host folds into Wqk;  |16*g|  < ~40
EXP_SCALE = 1.0 / (YSCALE * GSCALE * 32.0)  # exp((S_psum)/8192)


def build_program(nq=NQ, nk=NK, d=D, f=F, nblk=512):
    nc = bacc.Bacc(trn_type="TRN2")

    DC = d // P            # contraction chunks (8)
    MC = nk // P           # key chunks (16)
    NB = nq // nblk        # query blocks (4)
    NSUB = nblk // P       # 128-row subblocks per query block (4)
    FB = f // 512          # value free blocks (2)

    # Inputs are pre-arranged on host into exact SBUF layout [128, ...] so
    # every input DMA is a contiguous per-partition blit (multi-KB lines).
    qT = nc.dram_tensor("qT", [NB, P, DC * nblk], F16, kind="ExternalInput").ap()
    yT = nc.dram_tensor("yT", [P, DC * nk], F16, kind="ExternalInput").ap()
    y8T = nc.dram_tensor("y8T", [P, DC * nk], FP8, kind="ExternalInput").ap()
    Wqk = nc.dram_tensor("Wqk", [P, DC * d], F16, kind="ExternalInput").ap()
    Wkv = nc.dram_tensor("Wkv", [P, DC * f], F16, kind="ExternalInput").ap()
    out = nc.dram_tensor("out", [nq, f], F16, kind="ExternalOutput").ap()

    qT_v = qT.rearrange("b p (c n) -> b p c n", c=DC)
    yT_v = yT.rearrange("p (c m) -> p c m", c=DC)
    y8_v = y8T.rearrange("p (c m) -> p c m", c=DC)
    # Wqk host layout is e-chunk-major [p, ei, di, el] so the g-phase can
    # start on piece ei=0 after ~256KB of DMA instead of the full 2MB.
    Wqk_v = Wqk.rearrange("p (e c l) -> p e c l", e=DC, c=DC)
    Wkv_v = Wkv.rearrange("p (c f) -> p c f", c=DC)
    out_v = out.rearrange("(b p) f -> b p f", p=P)

    with tile.TileContext(nc) as tc, ExitStack() as ctx:
        consts = ctx.enter_context(tc.tile_pool(name="consts", bufs=1))
        y8_pool = ctx.enter_context(tc.tile_pool(name="y8", bufs=1))
        wqk_pool = ctx.enter_context(tc.tile_pool(name="wqk", bufs=1))
        v_pool = ctx.enter_context(tc.tile_pool(name="vproj", bufs=1))
        qt_pool = ctx.enter_context(tc.tile_pool(name="qt", bufs=4))
        g8_pool = ctx.enter_context(tc.tile_pool(name="g8", bufs=2))
        eT_pool = ctx.enter_context(tc.tile_pool(name="eT", bufs=2))
        red_pool = ctx.enter_context(tc.tile_pool(name="red", bufs=2))
        out_pool = ctx.enter_context(tc.tile_pool(name="outsb", bufs=4))
        small = ctx.enter_context(tc.tile_pool(name="small", bufs=8))
        yt_pool = ctx.enter_context(tc.tile_pool(name="yt", bufs=1))
        wkv_pool = ctx.enter_context(tc.tile_pool(name="wkv", bufs=1))
        psum_a = ctx.enter_context(
            tc.tile_pool(name="psum_a", bufs=4, space="PSUM"))
        psum_o = ctx.enter_context(
            tc.tile_pool(name="psum_o", bufs=3, space="PSUM"))
        psum_d = ctx.enter_context(
            tc.tile_pool(name="psum_d", bufs=1, space="PSUM"))

        ones16 = consts.tile([P, 1], F16)
        nc.vector.memset(ones16, 1.0)
        zbias = consts.tile([P, 1], F32)
        nc.vector.memset(zbias, 0.0)

        y8 = y8_pool.tile([P, DC, nk], FP8)       # [d_p, d_c, m]
        wqk = wqk_pool.tile([P, DC, DC, P], F16)  # [d_p, e_c, d_c, e_l]
        v_sb = v_pool.tile([P, MC, f], F16)       # [m_p, m_c, f]
        yt = yt_pool.tile([P, DC, nk], F16)       # [d_p, d_c, m]
        wkv = wkv_pool.tile([P, DC, f], F16)
        warm = consts.tile([P, 512], F16)
        nc.vector.memset(warm, 0.0)

        # ---- preload DMAs, striped across three queues in exact
        # consumption order: qt0 + wqk[0:4] first (g0 starts after
        # ~1.3MB), then the rest of wqk, y8 (S0), qt1 (g1), yt + wkv (V),
        # qt2/qt3 (bufs=4 so no WAR wait). ----
        qts = [qt_pool.tile([P, DC, nblk], F16, tag="qt", name=f"qt{b}")
               for b in range(NB)]
        nc.sync.dma_start(wqk[:, 0, :DC // 2], Wqk_v[:, 0, :DC // 2])
        nc.gpsimd.dma_start(wqk[:, 1, :DC // 2], Wqk_v[:, 1, :DC // 2])
        nc.sync.dma_start(wqk[:, 0, DC // 2:], Wqk_v[:, 0, DC // 2:])
        nc.gpsimd.dma_start(wqk[:, 1, DC // 2:], Wqk_v[:, 1, DC // 2:])
        for c in range(0, DC // 2, 2):
            nc.gpsimd.dma_start(qts[0][:, c:c + 2, :], qT_v[0][:, c:c + 2, :])
        for c in range(DC // 2, DC, 2):
            nc.sync.dma_start(qts[0][:, c:c + 2, :], qT_v[0][:, c:c + 2, :])
        for ei in range(2, DC):
            q_ = nc.sync if ei % 2 == 0 else nc.gpsimd
            q_.dma_start(wqk[:, ei], Wqk_v[:, ei])
        nc.sync.dma_start(y8[:, :DC // 2, :], y8_v[:, :DC // 2, :])
        nc.gpsimd.dma_start(y8[:, DC // 2:, :], y8_v[:, DC // 2:, :])
        nc.sync.dma_start(qts[1][:, :DC // 2, :], qT_v[1][:, :DC // 2, :])
        nc.gpsimd.dma_start(qts[1][:, DC // 2:, :], qT_v[1][:, DC // 2:, :])
        nc.sync.dma_start(yt[:, :DC // 2, :], yT_v[:, :DC // 2, :])
        nc.gpsimd.dma_start(yt[:, DC // 2:, :], yT_v[:, DC // 2:, :])
        nc.sync.dma_start(wkv[:, :DC // 2, :], Wkv_v[:, :DC // 2, :])
        nc.gpsimd.dma_start(wkv[:, DC // 2:, :], Wkv_v[:, DC // 2:, :])
        for b in range(2, NB):
            nc.sync.dma_start(qts[b][:, :DC // 2, :], qT_v[b][:, :DC // 2, :])
            nc.gpsimd.dma_start(qts[b][:, DC // 2:, :], qT_v[b][:, DC // 2:, :])

        # warm up the tensor engine p-state while the first DMAs land (the
        # clock needs ~3us of continuous execution to reach full speed, and
        # the first ~2.5MB of input takes ~8us to arrive anyway)
        for _ in range(16):
            wps = psum_a.tile([P, 512], F32, tag="psa", name="warm")
            nc.tensor.matmul(wps, lhsT=warm[:, 0:P], rhs=warm,
                             start=True, stop=True)

        def g_phase(qt, di_order=None):
            # di accumulation order is free; g0 consumes qt0 quarters in
            # DMA-arrival order (the two queues deliver di 0-1/4-5 first)
            order = di_order or list(range(DC))
            g8 = g8_pool.tile([P, DC, nblk], FP8, tag="g8", name="g8")
            for ei in range(DC):
                ps = psum_a.tile([P, 512], F32, tag="psa", name="psa")
                for k, di in enumerate(order):
                    nc.tensor.matmul(
                        ps,
                        lhsT=wqk[:, ei, di, :],
                        rhs=qt[:, di, :],
                        start=(k == 0), stop=(k == DC - 1))
                nc.scalar.activation(g8[:, ei, :], ps,
                                     mybir.ActivationFunctionType.Copy)
            return g8

        def s_phase(g8):
            # S[m, n] (psum = 256*scores_raw) -> eT = exp(psum/8192), fp16
            eT = eT_pool.tile([P, MC, nblk], F16, tag="eT", name="eT")
            for mi in range(MC):
                ps = psum_a.tile([P, 512], F32, tag="psa", name="psa")
                for c in range(DC // 2):
                    nc.tensor.matmul(
                        ps,
                        lhsT=y8[:, 2 * c:2 * c + 2, mi * P:(mi + 1) * P],
                        rhs=g8[:, 2 * c:2 * c + 2, :],
                        start=(c == 0), stop=(c == DC // 2 - 1),
                        perf_mode=mybir.MatmulPerfMode.DoubleRow)
                nc.scalar.activation(
                    eT[:, mi, :], ps,
                    mybir.ActivationFunctionType.Exp,
                    bias=zbias, scale=EXP_SCALE)
            # red[m_p, n] = sum_mi eT  (VectorE fp16 add chain, contiguous
            # slices, off the tensor engine; fp16 keeps the later 1-col
            # denominator matmul in the PE's fast clock domain -- an f32
            # matmul downclocks the whole PE by 20%)
            red = red_pool.tile([P, nblk], F16, tag="red", name="red")
            nc.vector.tensor_add(red, eT[:, 0, :], eT[:, 1, :])
            for mi in range(2, MC):
                nc.vector.tensor_add(red, red, eT[:, mi, :])
            return eT, red

        def v_phase():
            # v[m, f] = sum_d yT[d, m] * Wkv[d, f]  (fp16); psum drains
            # alternate VectorE/ScalarE so neither engine bottlenecks
            for fb in range(FB):
                for mi in range(MC):
                    ps = psum_a.tile([P, 512], F32, tag="psa", name="psa")
                    for di in range(DC):
                        nc.tensor.matmul(
                            ps,
                            lhsT=yt[:, di, mi * P:(mi + 1) * P],
                            rhs=wkv[:, di, fb * 512:(fb + 1) * 512],
                            start=(di == 0), stop=(di == DC - 1))
                    dst = v_sb[:, mi, fb * 512:(fb + 1) * 512]
                    if mi % 2 == 0:
                        nc.vector.tensor_copy(dst, ps)
                    else:
                        nc.scalar.copy(dst, ps)

        def out_phase(nb, eT, red, last=False):
            # out[n, f] = (eT.T @ v) / denom.  denom comes from red via one
            # 1-col f32 matmul per subblock.  fb0's 16-mi group runs first,
            # so its reciprocal + ScalarE scale + DMA overlap fb1's
            # matmuls; fb1 scales on VectorE.
            for ns in range(NSUB):
                lhs = [eT[:, mi, ns * P:(ns + 1) * P] for mi in range(MC)]
                pss = psum_d.tile([P, 1], F32, tag="pss", name="pss")
                nc.tensor.matmul(pss, lhsT=red[:, ns * P:(ns + 1) * P],
                                 rhs=ones16, start=True, stop=True)
                rec = small.tile([P, 1], F32)
                nc.vector.reciprocal(rec, pss)
                pos0 = psum_o.tile([P, 512], F32, tag="pso", name="pso")
                for mi in range(MC):
                    nc.tensor.matmul(
                        pos0, lhsT=lhs[mi], rhs=v_sb[:, mi, 0:512],
                        start=(mi == 0), stop=(mi == MC - 1))
                # emit fb0's scale + DMA BEFORE fb1's matmuls: cross-engine
                # deps are engine-counter waits derived from emission order,
                # so emitting later would chain them behind fb1's matmuls
                ob = out_pool.tile([P, f], F16, tag="ob", name="ob")
                nc.scalar.mul(ob[:, 0:512], pos0, rec)
                nc.sync.dma_start(
                    out_v[nb * NSUB + ns][:, 0:512], ob[:, 0:512])
                pos1 = psum_o.tile([P, 512], F32, tag="pso", name="pso")
                for mi in range(MC):
                    nc.tensor.matmul(
                        pos1, lhsT=lhs[mi], rhs=v_sb[:, mi, 512:1024],
                        start=(mi == 0), stop=(mi == MC - 1))
                if last and ns == NSUB - 1:
                    # split the final scale across both engines + 2 DMAs to
                    # shorten the end-of-kernel drain chain
                    nc.scalar.mul(ob[:, 512:768], pos1[:, 0:256], rec)
                    nc.vector.tensor_scalar_mul(
                        ob[:, 768:1024], pos1[:, 256:512], rec)
                    nc.gpsimd.dma_start(
                        out_v[nb * NSUB + ns][:, 512:768], ob[:, 512:768])
                    nc.sync.dma_start(
                        out_v[nb * NSUB + ns][:, 768:1024], ob[:, 768:1024])
                else:
                    nc.vector.tensor_scalar_mul(
                        ob[:, 512:1024], pos1, rec)
                    nc.sync.dma_start(
                        out_v[nb * NSUB + ns][:, 512:1024], ob[:, 512:1024])

        # ---- phase schedule: g0 S0 g1 S1 V out0 g2 S2 out1 g3 S3 out2
        # out3 (V deferred off the BW-bound startup stream) ----
        g8_0 = g_phase(qts[0], di_order=[0, 1, 4, 5, 2, 3, 6, 7])
        eT_0, red_0 = s_phase(g8_0)
        g8_1 = g_phase(qts[1])
        eT_1, red_1 = s_phase(g8_1)
        v_phase()
        out_phase(0, eT_0, red_0)
        g8_2 = g_phase(qts[2])
        eT_2, red_2 = s_phase(g8_2)
        out_phase(1, eT_1, red_1)
        g8_3 = g_phase(qts[3])
        eT_3, red_3 = s_phase(g8_3)
        out_phase(2, eT_2, red_2)
        out_phase(3, eT_3, red_3, last=True)

    nc.compile()
    return nc


def _sbufize(xT):
    """[d, X] row-major -> SBUF-layout blob [128, (d//128)*X] so the DMA is
    a contiguous per-partition blit."""
    dd, X = xT.shape
    c = dd // P
    return np.ascontiguousarray(
        xT.reshape(c, P, X).transpose(1, 0, 2).reshape(P, c * X))


def make_in_maps(q, y, Wq, Wk, Wv):
    """Host prep: weight products, transposes, dtype casts, fp8 quantize."""
    q = np.asarray(q, dtype=np.float32)
    y = np.asarray(y, dtype=np.float32)
    Wq = np.asarray(Wq, dtype=np.float32)
    Wk = np.asarray(Wk, dtype=np.float32)
    Wv = np.asarray(Wv, dtype=np.float32)

    # Wqk: e-chunk-major SBUF layout [p, ei, di, el]
    Wqk16 = (GSCALE * (Wq @ Wk.T)).astype(F16NP)      # [d, e]
    Wqk = np.ascontiguousarray(
        Wqk16.reshape(8, P, 8, P).transpose(1, 2, 0, 3).reshape(P, 8 * 1024))
    Wkv = _sbufize((Wk @ Wv).astype(F16NP))

    in_maps = []
    for b in range(B):
        qT = q[b].T.astype(F16NP)          # [1024, 2048]
        yT = y[b].T
        # per-block SBUF layout: [NB, 128, DC*nblk]
        qTb = np.ascontiguousarray(
            qT.reshape(8, P, 4, 512).transpose(2, 1, 0, 3).reshape(4, P, 8 * 512))
        in_maps.append({
            "qT": qTb,
            "yT": _sbufize(yT.astype(F16NP)),
            "y8T": _sbufize((YSCALE * yT).astype(E4NP)),
            "Wqk": Wqk, "Wkv": Wkv,
        })
    return in_maps


_CACHE = {}


def kernel(q, y, Wq, Wk, Wv):
    if "nc" not in _CACHE:
        _CACHE["nc"] = build_program()
    nc = _CACHE["nc"]
    in_maps = make_in_maps(q, y, Wq, Wk, Wv)
    res = run_bass_kernel_spmd(nc, in_maps, core_ids=list(range(B)))
    return np.stack(
        [res.results[b]["out"].astype(np.float32) for b in range(B)], axis=0)
